# revision 1
# baseline (speedup 1.0000x reference)
"""
Trainium2 Bass kernel for DirectRankingModel:
    h = tanh(x @ W1.T + b1); s = (h @ W2.T + b2); e = exp(s)
    out = e / segment_sum(e, T)[T]    with 2 segments, N = 2,000,000 rows.

Strategy (8 NeuronCores, data-parallel over rows):
  - Host: block-transpose x into [nblk, 64 feat, 128 rows] so each DMA
    descriptor moves contiguous 512B runs and the PE receives the
    feature-on-partition (transposed) operand directly.  Host also builds
    f32 masks m0/m1 = (T==0)/(T==1) (zero on padded rows).
  - Device per core (R = 262144 rows, padded; 8 super-tiles of 128x256):
      * SWDGE DMA with f32->f16 cast loads "xx" mega tiles [128, 2048]:
        partitions = (half, feature), free = rows.
      * mm1: two K=64 matmuls per 1024 rows (row-split PE: partitions 0-63
        and 64-127 run concurrently), W1T stationary -> PSUM hT [128h, 1024r].
      * tanh on the scalar engine with fused +b1 bias, PSUM -> SBUF fp16.
      * mm2: score s = W2 . h per row, laid out as [128 blocks, 256 rows]:
        32 strip matrices [128, 32] with W2 embedded in column c accumulate
        block b's scores into PSUM partition b (avoids a [1, N] layout).
      * exp with fused +b2 bias -> E [128, 2048] f32 stays SBUF-resident.
      * masked sums via tensor_tensor_reduce, partition_all_reduce, then a
        2-float HBM AllReduce across the 8 cores.
      * normalize: out = E * (inv0 + m1*(inv1-inv0)) and DMA out.
"""

import os
import sys

import numpy as np

for _p in ("/opt/trn_rl_repo", "/root/.axon_site/_ro/trn_rl_repo"):
    if os.path.isdir(_p) and _p not in sys.path:
        sys.path.insert(0, _p)

import concourse.bacc as bacc
import concourse.bass as bass
import concourse.tile as tile
from concourse import bass_isa, mybir
from concourse.bass_utils import run_bass_kernel_spmd

F16 = mybir.dt.float16
F32 = mybir.dt.float32
ALU = mybir.AluOpType
ACTF = mybir.ActivationFunctionType

N_CORES = 8
N_ROWS = 2_000_000
IN_DIM = 64
HID = 128

# Device-side geometry (per core).
Q = 256                 # rows per score-block (mm2 moving free dim)
N_ST = 8                # super-tiles per core; ST = 128 blocks x Q rows = 32768
MEGA_BLK = 32           # x blocks (128 rows each) per mega DMA tile -> 4096 rows
R_CORE = N_ST * 128 * Q          # 262144 rows per core
NBLK_CORE = R_CORE // 128        # 2048
N_PAD = N_CORES * R_CORE         # 2097152 rows total (padded)
NBLK_TOT = N_PAD // 128          # 16384
NBLK_REAL = N_ROWS // 128        # 15625

_MEGAS_PER_ST = (128 * Q) // (MEGA_BLK * 128)   # 8
_SUB_PER_MEGA = (MEGA_BLK * 128) // 1024        # 4  (1024-row mm1 pairs)
_BLOCKS_PER_MEGA = (MEGA_BLK * 128) // Q        # 16 (mm2 blocks per mega)


def _ap(handle_ap, offset, dims):
    """Custom access pattern on a DRAM tensor: dims = [[step, count], ...]."""
    return bass.AP(tensor=handle_ap.tensor, offset=offset, ap=list(dims))


def build_nc(n_st=N_ST, n_cores=N_CORES, use_coll=True, stage=9):
    """Build the per-core Bass program (SPMD: same program, sliced inputs)."""
    from contextlib import ExitStack

    r_core = n_st * 128 * Q
    nblk = r_core // 128
    cols = n_st * Q            # E/mask/out columns per partition

    n_mega = r_core // (MEGA_BLK * 128)

    nc = bacc.Bacc(num_devices=n_cores)

    x_in = nc.declare_dram_parameter(
        "x", [n_mega, 128, MEGA_BLK * 64], F32, isOutput=False
    )
    m0_in = nc.declare_dram_parameter("m0", [r_core], F32, isOutput=False)
    m1_in = nc.declare_dram_parameter("m1", [r_core], F32, isOutput=False)
    w1t_in = nc.declare_dram_parameter("w1t", [IN_DIM, HID], F16, isOutput=False)
    w2s_in = nc.declare_dram_parameter("w2s", [HID, 32 * 32], F16, isOutput=False)
    b1_in = nc.declare_dram_parameter("b1", [HID], F32, isOutput=False)
    b2_in = nc.declare_dram_parameter("b2", [1], F32, isOutput=False)
    out_t = nc.declare_dram_parameter("out", [r_core], F32, isOutput=True)
    gs_t = nc.declare_dram_parameter("gsums", [2], F32, isOutput=True)

    cc_in = nc.dram_tensor("cc_in", [2], F32)
    cc_out = nc.dram_tensor("cc_out", [2], F32, addr_space="Shared")

    B_ELEM = IN_DIM * 128  # elements per x block

    with ExitStack() as ctx:
        tc = ctx.enter_context(tile.TileContext(nc))
        singles = ctx.enter_context(tc.tile_pool(name="singles", bufs=1))
        xx_pool = ctx.enter_context(tc.tile_pool(name="xx", bufs=3))
        ht_pool = ctx.enter_context(tc.tile_pool(name="ht", bufs=3))
        ph_pool = ctx.enter_context(tc.tile_pool(name="ph", bufs=3, space="PSUM"))
        ps_pool = ctx.enter_context(tc.tile_pool(name="ps", bufs=1, space="PSUM"))

        # ---- static setup ----------------------------------------------
        w1t_sb = singles.tile([128, HID], F16)     # both halves hold W1T
        nc.sync.dma_start(
            out=w1t_sb[:],
            in_=_ap(w1t_in[:], 0, [[0, 2], [HID, IN_DIM], [1, HID]]),
        )
        b1_sb = singles.tile([128, 1], F32)
        nc.sync.dma_start(out=b1_sb[:], in_=_ap(b1_in[:], 0, [[1, HID], [1, 1]]))
        b2_sb = singles.tile([128, 1], F32)
        nc.sync.dma_start(out=b2_sb[:], in_=_ap(b2_in[:], 0, [[0, 128], [1, 1]]))

        # 32 strip matrices [128, 32] fp16, strip c has W2 in column c.
        strips = singles.tile([128, 32, 32], F16)
        nc.sync.dma_start(
            out=strips[:], in_=_ap(w2s_in[:], 0, [[32 * 32, HID], [1, 32 * 32]])
        )

        # Masks + persistent E (all f32, SBUF-resident for the whole kernel).
        m0_sb = singles.tile([128, cols], F32)
        m1_sb = singles.tile([128, cols], F32)
        mask_dims = [[Q, 128], [128 * Q, n_st], [1, Q]]
        nc.sync.dma_start(out=m0_sb[:], in_=_ap(m0_in[:], 0, mask_dims))
        nc.sync.dma_start(out=m1_sb[:], in_=_ap(m1_in[:], 0, mask_dims))
        e_sb = singles.tile([128, cols], F32)
        scratch = singles.tile([128, cols], F32)
        out_sb = singles.tile([128, cols], F32)
        rr = singles.tile([128, 2], F32)
        rr_red = singles.tile([128, 2], F32)
        ones_sb = singles.tile([128, 1], F32)
        nc.vector.memset(ones_sb[:], 1.0)
        g_sb = singles.tile([128, 2], F32)
        inv = singles.tile([128, 2], F32)
        dinv = singles.tile([128, 1], F32)

        # ---- phase 1: matmuls / tanh / scores / exp --------------------
        for st in range(n_st):
            s_ps = ps_pool.tile([128, Q], F32, tag="score")
            for m in range(_MEGAS_PER_ST):
                mega = st * _MEGAS_PER_ST + m
                half = MEGA_BLK * 64  # 2048 rows: partition halves g=0/1
                xx = xx_pool.tile([128, half], F16, tag="xx")
                src = _ap(
                    x_in[:],
                    mega * 128 * half,
                    [[half, 128], [1, half]],
                )
                nc.gpsimd.dma_start(out=xx[:], in_=src)  # f32 -> f16 cast DMA

                ht = ht_pool.tile([128, MEGA_BLK * 128], F16, tag="ht")
                for t in range(_SUB_PER_MEGA):
                    ph = ph_pool.tile([128, 1024], F32, tag="ph")
                    nc.tensor.matmul(
                        ph[:, 0:512],
                        w1t_sb[0:64, :],
                        xx[0:64, t * 512 : (t + 1) * 512],
                        start=True,
                        stop=True,
                    )
                    nc.tensor.matmul(
                        ph[:, 512:1024],
                        w1t_sb[64:128, :],
                        xx[64:128, t * 512 : (t + 1) * 512],
                        start=True,
                        stop=True,
                    )
                    # ht col layout is (t, g, j): col = t*1024 + g*512 + j,
                    # holding row mega_base + g*2048 + t*512 + j.
                    nc.scalar.activation(
                        out=ht[:, t * 1024 : (t + 1) * 1024],
                        in_=ph[:, 0:1024],
                        func=ACTF.Tanh,
                        bias=b1_sb[:],
                        scale=1.0,
                    )
                for bl in range(_BLOCKS_PER_MEGA):
                    b = m * _BLOCKS_PER_MEGA + bl
                    c = b % 32
                    g = b // 32
                    # rows bl*256..+256 of this mega live at ht col offset:
                    hoff = ((bl % 8) // 2) * 1024 + (bl // 8) * 512 + (bl % 2) * Q
                    nc.tensor.matmul(
                        s_ps[32 * g : 32 * g + 32, :],
                        strips[:, c, :],
                        ht[:, hoff : hoff + Q],
                        start=(c == 0),
                        stop=(c == 31),
                        skip_group_check=True,
                        tile_position=(0, 32 * g),
                    )
            nc.scalar.activation(
                out=e_sb[:, st * Q : (st + 1) * Q],
                in_=s_ps[:],
                func=ACTF.Exp,
                bias=b2_sb[:],
                scale=1.0,
            )

        # ---- segment sums + allreduce ----------------------------------
        if stage <= 1:
            # phase-1 only: dump E and a dummy gsums
            nc.sync.dma_start(
                out=_ap(out_t[:], 0, [[Q, 128], [128 * Q, n_st], [1, Q]]),
                in_=e_sb[:],
            )
            nc.sync.dma_start(out=gs_t[:], in_=e_sb[0:1, 0:2])
            nc.compile()
            return nc
        nc.vector.tensor_mul(scratch[:], e_sb[:], m0_sb[:])
        nc.vector.reduce_sum(rr[:, 0:1], scratch[:], axis=mybir.AxisListType.X)
        nc.vector.tensor_mul(scratch[:], e_sb[:], m1_sb[:])
        nc.vector.reduce_sum(rr[:, 1:2], scratch[:], axis=mybir.AxisListType.X)
        if stage <= 2:
            # skip partition reduce: use per-partition sums (wrong values)
            nc.vector.tensor_copy(rr_red[:], rr[:])
        else:
            # cross-partition sum via ones-matmul (PE), [128,2] -> [1,2]
            ps_rr = ps_pool.tile([128, 2], F32, tag="score")
            nc.tensor.matmul(
                ps_rr[0:1, :], ones_sb[:], rr[:], start=True, stop=True
            )
            nc.scalar.activation(
                out=rr_red[0:1, :],
                in_=ps_rr[0:1, :],
                func=ACTF.Copy,
                bias=0.0,
                scale=1.0,
            )
        if use_coll:
            nc.gpsimd.dma_start(out=cc_in[:], in_=rr_red[0:1, :])
            nc.gpsimd.collective_compute(
                "AllReduce",
                ALU.add,
                replica_groups=[list(range(n_cores))],
                ins=[cc_in[:]],
                outs=[cc_out[:]],
            )
            nc.sync.dma_start(out=gs_t[:], in_=cc_out[:])
            nc.sync.dma_start(
                out=g_sb[:], in_=_ap(cc_out[:], 0, [[0, 128], [1, 2]])
            )
        else:
            nc.sync.dma_start(out=gs_t[:], in_=rr_red[0:1, :])
            nc.vector.tensor_copy(g_sb[:], rr_red[:])

        # ---- normalize + store -----------------------------------------
        nc.vector.reciprocal(out=inv[:], in_=g_sb[:])
        nc.vector.tensor_sub(dinv[:], inv[:, 1:2], inv[:, 0:1])
        nc.vector.tensor_scalar(
            out=scratch[:],
            in0=m1_sb[:],
            scalar1=dinv[:],
            scalar2=inv[:, 0:1],
            op0=ALU.mult,
            op1=ALU.add,
        )
        nc.vector.tensor_mul(out_sb[:], scratch[:], e_sb[:])
        nc.sync.dma_start(
            out=_ap(out_t[:], 0, [[Q, 128], [128 * Q, n_st], [1, Q]]),
            in_=out_sb[:],
        )

    nc.compile()
    return nc


_NC_CACHE = {}


def _get_nc(n_st=N_ST):
    if n_st not in _NC_CACHE:
        _NC_CACHE[n_st] = build_nc(n_st=n_st)
    return _NC_CACHE[n_st]


def prep_inputs(x, T, W1, b1, W2, b2, n_st=N_ST, n_cores=N_CORES):
    """Host-side shard/layout prep -> per-core input maps."""
    r_core = n_st * 128 * Q
    nblk = r_core // 128
    n_pad = n_cores * r_core
    n_rows = x.shape[0]
    nblk_real = n_rows // 128

    x = np.ascontiguousarray(np.asarray(x, dtype=np.float32))
    rows_mega = MEGA_BLK * 128                      # 4096
    half = rows_mega // 2                           # 2048
    n_mega_tot = n_pad // rows_mega
    n_full = n_rows // rows_mega
    xd = np.zeros((n_mega_tot, 128, half), dtype=np.float32)
    xd[:n_full] = (
        x[: n_full * rows_mega]
        .reshape(n_full, 2, half, IN_DIM)
        .transpose(0, 1, 3, 2)
        .reshape(n_full, 128, half)
    )
    rem = n_rows - n_full * rows_mega
    if rem:
        r0 = min(rem, half)
        xd[n_full, :IN_DIM, :r0] = x[n_full * rows_mega :][:r0].T
        if rem > half:
            xd[n_full, IN_DIM:, : rem - half] = x[n_full * rows_mega + half :].T
    n_mega_core = n_mega_tot // n_cores

    T = np.asarray(T)
    m0 = np.zeros(n_pad, dtype=np.float32)
    m1 = np.zeros(n_pad, dtype=np.float32)
    m0[:n_rows] = T == 0
    m1[:n_rows] = T == 1

    w1t = np.ascontiguousarray(np.asarray(W1, np.float32).T).astype(np.float16)
    w2s = np.zeros((HID, 32, 32), dtype=np.float16)
    w2v = np.asarray(W2, np.float32).reshape(HID).astype(np.float16)
    for c in range(32):
        w2s[:, c, c] = w2v
    w2s = w2s.reshape(HID, 32 * 32)
    b1h = np.asarray(b1, np.float32).reshape(HID).copy()
    b2h = np.asarray(b2, np.float32).reshape(1).copy()

    in_maps = []
    for cid in range(n_cores):
        in_maps.append(
            {
                "x": xd[cid * n_mega_core : (cid + 1) * n_mega_core],
                "m0": m0[cid * r_core : (cid + 1) * r_core],
                "m1": m1[cid * r_core : (cid + 1) * r_core],
                "w1t": w1t,
                "w2s": w2s,
                "b1": b1h,
                "b2": b2h,
            }
        )
    return in_maps


def run(x, T, W1, b1, W2, b2, n_st=N_ST, trace=False):
    in_maps = prep_inputs(x, T, W1, b1, W2, b2, n_st=n_st)
    nc = _get_nc(n_st)
    res = run_bass_kernel_spmd(nc, in_maps, list(range(N_CORES)), trace=trace)
    out = np.concatenate([res.results[c]["out"] for c in range(N_CORES)])
    return out[: x.shape[0]].astype(np.float32, copy=False), res


def kernel(x, T, W1, b1, W2, b2):
    out, _ = run(x, T, W1, b1, W2, b2)
    return out



# revision 22
# speedup vs baseline: 1.0519x; 1.0519x over previous
"""
Trainium2 Bass kernel for DirectRankingModel:
    h = tanh(x @ W1.T + b1); s = (h @ W2.T + b2); e = exp(s)
    out = e / segment_sum(e, T)[T]    with 2 segments, N = 2,000,000 rows.

Strategy (8 NeuronCores, data-parallel over rows; measured 334 us vs 355 us
for the v1 baseline; the scalar engine's tanh stream is the bottleneck and
runs at ~100% occupancy during the main phase):
  - Host: cast x to fp16 and block-transpose into [nmega, 128, 2048] so each
    DMA moves 4KB contiguous runs. fp16 halves the HBM read (512->256 MB)
    vs the f32+cast-DMA baseline, taking DMA off the critical path.
    Host also builds fp16 masks m0/m1 = (T==0)/(T==1), zero on padded rows.
  - Device per core (R = 262144 rows; 4 super-tiles of 128 x 512 scores):
      * x loads ride the sync ring (the gpsimd SWDGE ring wedges the device
        if shared with remote-DMA descriptors; see rdma_ar note below).
      * mm1: two K=64 matmuls per 1024 rows (row-split PE), W1T stationary
        -> PSUM [128h, 1024r], 4 sub-tiles per 4096-row mega.
      * tanh on the scalar engine (fused +b1 bias), PSUM -> SBUF fp16.
        (A DVE/Pade offload was tried and measured SLOWER: no 2x DVE perf
        modes engage for tensor_tensor/stt on this silicon, so the 7-pass
        rational costs ~7.7 ns/elem vs scalar's 1.0 -- see dve_mod flag.)
      * mm2: 32 strip matrices [128, 32] with W2 embedded in column c
        accumulate block scores into PSUM [128, 512] per super-tile
        (Q=512 halves LDWEIGHTS vs Q=256). mm2 for mega k is emitted after
        mm1 of mega k+2 so the in-order PE queue never stalls on a late ht.
      * exp (+b2) -> E [128, 2048] f32 SBUF-resident; masked group sums run
        INCREMENTALLY per super-tile on the DVE (mul + reduce_sum into
        per-ST accumulator columns), overlapped under the next super-tile.
      * tail: 3 adds + ones-matmul partition reduce + 2-float HBM AllReduce
        (collective_compute; ~33 us -- an SBUF remote-DMA allreduce works
        standalone (probe verified, XOR-slot broadcast) but wedges the
        device inside this full kernel under every structure tried
        (tile_critical and raw post-context); kept behind rdma_ar=False), reciprocal, scale = inv0+m1*(inv1-inv0),
        out = E*scale, scatter-DMA out.
"""

import os
import sys

import numpy as np

for _p in ("/opt/trn_rl_repo", "/root/.axon_site/_ro/trn_rl_repo"):
    if os.path.isdir(_p) and _p not in sys.path:
        sys.path.insert(0, _p)

import concourse.bacc as bacc
import concourse.bass as bass
import concourse.tile as tile
from concourse import bass_isa, mybir
from concourse.bass_utils import run_bass_kernel_spmd

F16 = mybir.dt.float16
BF16 = mybir.dt.bfloat16
F32 = mybir.dt.float32
ALU = mybir.AluOpType
ACTF = mybir.ActivationFunctionType

N_CORES = 8
N_ROWS = 2_000_000
IN_DIM = 64
HID = 128

# Device-side geometry (per core).
Q = 512                 # rows per score-block (mm2 moving free dim)
N_ST = 4                # super-tiles per core; ST = 128 blocks x Q rows = 65536
MEGA_BLK = 32           # x blocks (128 rows each) per mega DMA tile -> 4096 rows
R_CORE = N_ST * 128 * Q          # 262144 rows per core
N_PAD = N_CORES * R_CORE         # 2097152 rows total (padded)

ROWS_MEGA = MEGA_BLK * 128       # 4096
HALF = ROWS_MEGA // 2            # 2048 rows per partition-half
N_MEGA_CORE = R_CORE // ROWS_MEGA            # 64
MEGAS_PER_ST = (128 * Q) // ROWS_MEGA        # 16
BLOCKS_PER_MEGA = ROWS_MEGA // Q             # 8

DVE_MOD = 0             # 1 of every DVE_MOD [128,2048] tanh chunks -> DVE path
DVE_PHASE = 1


def _ap(handle_ap, offset, dims):
    """Custom access pattern on a DRAM tensor: dims = [[step, count], ...]."""
    return bass.AP(tensor=handle_ap.tensor, offset=offset, ap=list(dims))


def build_nc(n_cores=N_CORES, dve_mod=DVE_MOD, f16_masks=True, inc_sums=True,
             rdma_ar=False):
    """Build the per-core Bass program (SPMD: same program, sliced inputs)."""
    from contextlib import ExitStack

    MDT = F16 if f16_masks else F32
    n_st = N_ST
    cols = n_st * Q            # E/mask/out columns per partition (2048)
    n_mega = N_MEGA_CORE

    nc = bacc.Bacc(num_devices=n_cores)

    x_in = nc.declare_dram_parameter(
        "x", [n_mega, 128, HALF], F16, isOutput=False
    )
    m0_in = nc.declare_dram_parameter("m0", [R_CORE], MDT, isOutput=False)
    m1_in = nc.declare_dram_parameter("m1", [R_CORE], MDT, isOutput=False)
    w1t_in = nc.declare_dram_parameter("w1t", [IN_DIM, HID], F16, isOutput=False)
    w2s_in = nc.declare_dram_parameter("w2s", [HID, 32 * 32], F16, isOutput=False)
    b1_in = nc.declare_dram_parameter("b1", [HID], F32, isOutput=False)
    b2_in = nc.declare_dram_parameter("b2", [1], F32, isOutput=False)
    out_t = nc.declare_dram_parameter("out", [R_CORE], F32, isOutput=True)
    gs_t = nc.declare_dram_parameter("gsums", [2], F32, isOutput=True)

    cc_in = nc.dram_tensor("cc_in", [2], F32)
    cc_out = nc.dram_tensor("cc_out", [2], F32, addr_space="Shared")

    with ExitStack() as ctx:
        tc = ctx.enter_context(tile.TileContext(nc))
        singles = ctx.enter_context(tc.tile_pool(name="singles", bufs=1))
        xx_pool = ctx.enter_context(tc.tile_pool(name="xx", bufs=3))
        ht_pool = ctx.enter_context(tc.tile_pool(name="ht", bufs=4))
        ph_pool = ctx.enter_context(tc.tile_pool(name="ph", bufs=3, space="PSUM"))
        ps_pool = ctx.enter_context(tc.tile_pool(name="ps", bufs=2, space="PSUM"))
        y_pool = ctx.enter_context(tc.tile_pool(name="y", bufs=2))
        dv_pool = ctx.enter_context(tc.tile_pool(name="dv", bufs=1))

        # ---- static setup ----------------------------------------------
        w1t_sb = singles.tile([128, HID], F16)     # both halves hold W1T
        nc.sync.dma_start(
            out=w1t_sb[:],
            in_=_ap(w1t_in[:], 0, [[0, 2], [HID, IN_DIM], [1, HID]]),
        )
        b1_sb = singles.tile([128, 1], F32)
        nc.sync.dma_start(out=b1_sb[:], in_=_ap(b1_in[:], 0, [[1, HID], [1, 1]]))
        b2_sb = singles.tile([128, 1], F32)
        nc.sync.dma_start(out=b2_sb[:], in_=_ap(b2_in[:], 0, [[0, 128], [1, 1]]))

        # 32 strip matrices [128, 32] fp16, strip c has W2 in column c.
        # (loaded after the first mega's work is queued -- see mega loop)
        strips = singles.tile([128, 32, 32], F16)

        # Masks fp16, E f32, persistent SBUF tiles.
        m0_sb = singles.tile([128, cols], MDT)
        m1_sb = singles.tile([128, cols], MDT)
        mask_dims = [[Q, 128], [128 * Q, n_st], [1, Q]]
        e_sb = singles.tile([128, cols], F32)
        scratch = singles.tile([128, Q], F32)      # ttr full-tensor out (unused)
        scale_sb = singles.tile([128, cols], F32)
        out_sb = singles.tile([128, cols], F32)
        rr_accs = [
            singles.tile([128, 2], F32, name=f"rr_acc{i}") for i in range(2)
        ]
        rr_red = singles.tile([128, 2], F32)
        acc_sb = singles.tile([128, 2 * n_st], F32)
        ones_sb = singles.tile([128, 1], F32)
        nc.vector.memset(ones_sb[:], 1.0)
        ones_row = singles.tile([1, 128], F32)
        nc.vector.memset(ones_row[:], 1.0)
        g_sb = singles.tile([128, 2], F32)
        inv = singles.tile([128, 2], F32)
        dinv = singles.tile([128, 1], F32)

        # ---- phase 1: matmuls / tanh (scalar + DVE split) / scores / exp
        # mm2 for mega k is emitted after mm1 of mega k+LOOKAHEAD so a slow
        # DVE tanh chunk cannot stall the in-order PE queue.
        LOOKAHEAD = 2
        ht_tiles = {}
        s_ps_tiles = {}

        def emit_mm1(mega):
            xx = xx_pool.tile([128, HALF], F16, tag="xx")
            src = _ap(x_in[:], mega * 128 * HALF, [[HALF, 128], [1, HALF]])
            # sync ring: the gpsimd SWDGE ring is reserved for the RDMA
            # allreduce (sharing it with dma_start wedges the device)
            nc.sync.dma_start(out=xx[:], in_=src)

            ht = ht_pool.tile([128, ROWS_MEGA], F16, tag="ht")
            ht_tiles[mega] = ht
            for half in range(2):
                chunk = mega * 2 + half
                is_dve = dve_mod > 0 and (chunk % dve_mod) == DVE_PHASE
                if is_dve:
                    y = y_pool.tile([128, 2048], BF16, tag="y")
                for t2 in range(2):
                    t = half * 2 + t2
                    ph = ph_pool.tile([128, 1024], F32, tag="ph")
                    nc.tensor.matmul(
                        ph[:, 0:512],
                        w1t_sb[0:64, :],
                        xx[0:64, t * 512 : (t + 1) * 512],
                        start=True,
                        stop=True,
                    )
                    nc.tensor.matmul(
                        ph[:, 512:1024],
                        w1t_sb[64:128, :],
                        xx[64:128, t * 512 : (t + 1) * 512],
                        start=True,
                        stop=True,
                    )
                    # ht col layout is (t, g, j): col = t*1024 + g*512 + j,
                    # holding row mega_base + g*2048 + t*512 + j.
                    if is_dve:
                        # GPSIMD cannot read PSUM; bias+cast runs on DVE
                        nc.vector.tensor_scalar(
                            out=y[:, t2 * 1024 : (t2 + 1) * 1024],
                            in0=ph[:, 0:1024],
                            scalar1=b1_sb[:],
                            scalar2=None,
                            op0=ALU.add,
                        )
                    else:
                        nc.scalar.activation(
                            out=ht[:, t * 1024 : (t + 1) * 1024],
                            in_=ph[:, 0:1024],
                            func=ACTF.Tanh,
                            bias=b1_sb[:],
                            scale=1.0,
                        )
                if is_dve:
                    # Pade(5,4): h = y*(945+105u+u^2) / (945+420u+15u^2)
                    u = dv_pool.tile([128, 2048], BF16, tag="u")
                    n1 = dv_pool.tile([128, 2048], BF16, tag="n1")
                    nx = dv_pool.tile([128, 2048], BF16, tag="nx")
                    d1 = dv_pool.tile([128, 2048], BF16, tag="d1")
                    d2 = dv_pool.tile([128, 2048], F32, tag="d2")
                    rcp = dv_pool.tile([128, 2048], F32, tag="rcp")
                    nc.vector.tensor_tensor(
                        out=u[:], in0=y[:], in1=y[:], op=ALU.mult
                    )
                    nc.vector.scalar_tensor_tensor(
                        out=n1[:], in0=u[:], scalar=105.0, in1=u[:],
                        op0=ALU.add, op1=ALU.mult,
                    )
                    nc.vector.scalar_tensor_tensor(
                        out=nx[:], in0=n1[:], scalar=945.0, in1=y[:],
                        op0=ALU.add, op1=ALU.mult,
                    )
                    nc.vector.scalar_tensor_tensor(
                        out=d1[:], in0=u[:], scalar=28.0, in1=u[:],
                        op0=ALU.add, op1=ALU.mult,
                    )
                    nc.vector.tensor_scalar(
                        out=d2[:], in0=d1[:], scalar1=63.0, scalar2=15.0,
                        op0=ALU.add, op1=ALU.mult,
                    )
                    nc.vector.reciprocal_approx_fast(out=rcp[:], in_=d2[:])
                    ho = half * 2048
                    # final multiply on GPSIMD (SBUF-only) to unload DVE
                    nc.gpsimd.tensor_tensor(
                        out=ht[:, ho : ho + 2048], in0=nx[:], in1=rcp[:],
                        op=ALU.mult,
                    )

        def emit_mm2(mega):
            st = mega // MEGAS_PER_ST
            if mega % MEGAS_PER_ST == 0:
                s_ps_tiles[st] = ps_pool.tile(
                    [128, Q], F32, tag="score", name=f"s_ps{st}"
                )
            s_ps = s_ps_tiles[st]
            ht = ht_tiles.pop(mega)
            for b8 in range(BLOCKS_PER_MEGA):
                B = mega * BLOCKS_PER_MEGA + b8
                c = B % 32
                g = (B // 32) % 4
                hoff = (b8 % 4) * 1024 + (b8 // 4) * 512
                nc.tensor.matmul(
                    s_ps[32 * g : 32 * g + 32, :],
                    strips[:, c, :],
                    ht[:, hoff : hoff + Q],
                    start=(c == 0),
                    stop=(c == 31),
                    skip_group_check=True,
                    tile_position=(0, 32 * g),
                )
            if mega % MEGAS_PER_ST != MEGAS_PER_ST - 1:
                return
            # super-tile closed: exp + incremental masked sums
            s_ps = s_ps_tiles.pop(st)
            nc.scalar.activation(
                out=e_sb[:, st * Q : (st + 1) * Q],
                in_=s_ps[:],
                func=ACTF.Exp,
                bias=b2_sb[:],
                scale=1.0,
            )
            if inc_sums:
                e_sl = e_sb[:, st * Q : (st + 1) * Q]
                for gi, m_sb in enumerate((m0_sb, m1_sb)):
                    nc.vector.tensor_mul(
                        scratch[:], e_sl, m_sb[:, st * Q : (st + 1) * Q]
                    )
                    nc.vector.reduce_sum(
                        acc_sb[:, 2 * st + gi : 2 * st + gi + 1],
                        scratch[:],
                        axis=mybir.AxisListType.X,
                    )

        for mega in range(n_mega):
            emit_mm1(mega)
            if mega == 0:
                # deferred setup loads: off the first-tanh critical path
                nc.sync.dma_start(
                    out=strips[:],
                    in_=_ap(w2s_in[:], 0, [[32 * 32, HID], [1, 32 * 32]]),
                )
                nc.sync.dma_start(out=m0_sb[:], in_=_ap(m0_in[:], 0, mask_dims))
                nc.sync.dma_start(out=m1_sb[:], in_=_ap(m1_in[:], 0, mask_dims))
            if mega >= LOOKAHEAD:
                emit_mm2(mega - LOOKAHEAD)
        for mega in range(n_mega - LOOKAHEAD, n_mega):
            emit_mm2(mega)

        # ---- partition reduce + allreduce ------------------------------
        if inc_sums:
            rr_fin = rr_accs[1]
            nc.vector.tensor_add(rr_accs[0][:], acc_sb[:, 0:2], acc_sb[:, 2:4])
            nc.vector.tensor_add(rr_red[:], acc_sb[:, 4:6], acc_sb[:, 6:8])
            nc.vector.tensor_add(rr_fin[:], rr_accs[0][:], rr_red[:])
        else:
            rr_fin = rr_accs[0]
            nc.vector.tensor_mul(scale_sb[:], e_sb[:], m0_sb[:])
            nc.vector.reduce_sum(
                rr_fin[:, 0:1], scale_sb[:], axis=mybir.AxisListType.X
            )
            nc.vector.tensor_mul(scale_sb[:], e_sb[:], m1_sb[:])
            nc.vector.reduce_sum(
                rr_fin[:, 1:2], scale_sb[:], axis=mybir.AxisListType.X
            )
        if rdma_ar:
            # Partition-reduce the per-partition partials to [1, 2] and
            # broadcast back to all 128 partitions, so the cross-core
            # exchange operand is partition-uniform and the post-exchange
            # tail is pure vector work.
            rr_all = singles.tile([128, 16], F32)
            tmp8 = singles.tile([128, 8], F32)
            tmp4 = singles.tile([128, 4], F32)
            rr_g = singles.tile([128, 2], F32)
            rr_bcast = singles.tile([128, 2], F32)
            ps_rr = ps_pool.tile([128, Q], F32, tag="score")
            nc.tensor.matmul(
                ps_rr[0:1, 0:2], ones_sb[:], rr_fin[:], start=True, stop=True
            )
            nc.scalar.activation(
                out=rr_red[0:1, :],
                in_=ps_rr[0:1, 0:2],
                func=ACTF.Copy,
                bias=0.0,
                scale=1.0,
            )
            ps_bc = ps_pool.tile([128, Q], F32, tag="score", name="ps_bc")
            nc.tensor.matmul(
                ps_bc[:, 0:2],
                ones_row[:],
                rr_red[0:1, 0:2],
                start=True,
                stop=True,
            )
            nc.vector.tensor_copy(rr_bcast[:], ps_bc[:, 0:2])
        else:
            ps_rr = ps_pool.tile([128, Q], F32, tag="score")
            nc.tensor.matmul(
                ps_rr[0:1, 0:2], ones_sb[:], rr_fin[:], start=True, stop=True
            )
            nc.scalar.activation(
                out=rr_red[0:1, :],
                in_=ps_rr[0:1, 0:2],
                func=ACTF.Copy,
                bias=0.0,
                scale=1.0,
            )
            nc.gpsimd.dma_start(out=cc_in[:], in_=rr_red[0:1, :])
            nc.gpsimd.collective_compute(
                "AllReduce",
                ALU.add,
                replica_groups=[list(range(n_cores))],
                ins=[cc_in[:]],
                outs=[cc_out[:]],
            )
            nc.sync.dma_start(out=gs_t[:], in_=cc_out[:])
            nc.sync.dma_start(
                out=g_sb[:], in_=_ap(cc_out[:], 0, [[0, 128], [1, 2]])
            )

        if not rdma_ar:
            # ---- normalize + store (tile-scheduled) --------------------
            nc.vector.reciprocal(out=inv[:], in_=g_sb[:])
            nc.vector.tensor_sub(dinv[:], inv[:, 1:2], inv[:, 0:1])
            nc.vector.tensor_scalar(
                out=scale_sb[:],
                in0=m1_sb[:],
                scalar1=dinv[:],
                scalar2=inv[:, 0:1],
                op0=ALU.mult,
                op1=ALU.add,
            )
            nc.vector.tensor_mul(out_sb[:], scale_sb[:], e_sb[:])
            nc.sync.dma_start(
                out=_ap(out_t[:], 0, [[Q, 128], [128 * Q, n_st], [1, Q]]),
                in_=out_sb[:],
            )

    if rdma_ar:
        # ---- raw-bass tail after the TileContext teardown barrier ------
        # SBUF-to-SBUF allreduce: 8 broadcasts, each with ONE real
        # destination at XOR slot k (receiver r's slot k <- sender r^k;
        # identical SPMD program on every core). All cross-engine ordering
        # is manual; semaphores pinned far above the tile range. Tile APs
        # are rebuilt over the concrete SBUF tensors (the tile symbolic
        # form cannot lower outside the TileContext).
        def P(sym):
            return bass.AP(
                tensor=sym.tensor.concrete_tensor(),
                offset=sym.offset,
                ap=[list(d) for d in sym.ap],
            )

        rsem = nc.alloc_semaphore(name="ar_rsem", num=244)
        lsem = nc.alloc_semaphore(name="ar_lsem", num=245)
        psem = nc.alloc_semaphore(name="ar_psem", num=246)
        fsem = nc.alloc_semaphore(name="ar_fsem", num=247)
        dsem = nc.alloc_semaphore(name="ar_dsem", num=248)
        for k in range(n_cores):
            rdests = [None] * n_cores
            rdests[k] = (0, k)
            nc.gpsimd.remote_dma_broadcast(
                out_ap=P(rr_all[:, 2 * k : 2 * k + 2]),
                in_ap=P(rr_bcast[:]),
                remote_sem=rsem,
                local_sem=lsem,
                rdests=rdests,
            ).then_inc(psem, 1)
        nc.gpsimd.wait_ge(psem, n_cores)
        nc.gpsimd.trigger_dma(count=n_cores)
        # each of the 8 senders' arrivals bumps rsem by 16//8 = 2
        nc.vector.wait_ge(rsem, 2 * n_cores)
        for i in range(4):
            nc.vector.tensor_add(
                P(tmp8[:, 2 * i : 2 * i + 2]),
                P(rr_all[:, 2 * i : 2 * i + 2]),
                P(rr_all[:, 2 * i + 8 : 2 * i + 10]),
            )
        nc.vector.tensor_add(P(tmp4[:, 0:2]), P(tmp8[:, 0:2]), P(tmp8[:, 4:6]))
        nc.vector.tensor_add(P(tmp4[:, 2:4]), P(tmp8[:, 2:4]), P(tmp8[:, 6:8]))
        nc.vector.tensor_add(P(rr_g[:]), P(tmp4[:, 0:2]), P(tmp4[:, 2:4]))
        # normalize + store, all on vector then sync
        nc.vector.reciprocal(out=P(inv[:]), in_=P(rr_g[:]))
        nc.vector.tensor_sub(P(dinv[:]), P(inv[:, 1:2]), P(inv[:, 0:1]))
        nc.vector.tensor_scalar(
            out=P(scale_sb[:]),
            in0=P(m1_sb[:]),
            scalar1=P(dinv[:]),
            scalar2=P(inv[:, 0:1]),
            op0=ALU.mult,
            op1=ALU.add,
        )
        nc.vector.tensor_mul(
            P(out_sb[:]), P(scale_sb[:]), P(e_sb[:])
        ).then_inc(fsem, 1)
        nc.sync.wait_ge(fsem, 1)
        nc.sync.dma_start(out=gs_t[:], in_=P(rr_g[0:1, 0:2])).then_inc(
            dsem, 16
        )
        nc.sync.dma_start(
            out=_ap(out_t[:], 0, [[Q, 128], [128 * Q, n_st], [1, Q]]),
            in_=P(out_sb[:]),
        ).then_inc(dsem, 16)
        # flush everything, then clear our semaphores for re-execution
        nc.gpsimd.wait_ge(lsem, 16 * n_cores)
        nc.gpsimd.wait_ge(dsem, 32)
        for s in (rsem, lsem, psem, fsem, dsem):
            nc.gpsimd.sem_clear(s)

    nc.compile()
    return nc


_NC_CACHE = {}


def _get_nc(dve_mod=DVE_MOD, f16_masks=True, inc_sums=True, rdma_ar=False):
    key = (dve_mod, f16_masks, inc_sums, rdma_ar)
    if key not in _NC_CACHE:
        _NC_CACHE[key] = build_nc(
            dve_mod=dve_mod, f16_masks=f16_masks, inc_sums=inc_sums,
            rdma_ar=rdma_ar,
        )
    return _NC_CACHE[key]


def prep_inputs(x, T, W1, b1, W2, b2, n_cores=N_CORES, f16_masks=True):
    """Host-side shard/layout prep -> per-core input maps."""
    n_rows = x.shape[0]
    mdt = np.float16 if f16_masks else np.float32

    x16 = np.asarray(x, dtype=np.float32).astype(np.float16)
    n_mega_tot = N_PAD // ROWS_MEGA
    n_full = n_rows // ROWS_MEGA
    xd = np.zeros((n_mega_tot, 128, HALF), dtype=np.float16)
    xd[:n_full] = (
        x16[: n_full * ROWS_MEGA]
        .reshape(n_full, 2, HALF, IN_DIM)
        .transpose(0, 1, 3, 2)
        .reshape(n_full, 128, HALF)
    )
    rem = n_rows - n_full * ROWS_MEGA
    if rem:
        r0 = min(rem, HALF)
        xd[n_full, :IN_DIM, :r0] = x16[n_full * ROWS_MEGA :][:r0].T
        if rem > HALF:
            xd[n_full, IN_DIM:, : rem - HALF] = x16[n_full * ROWS_MEGA + HALF :].T
    n_mega_core = n_mega_tot // n_cores

    T = np.asarray(T)
    m0 = np.zeros(N_PAD, dtype=mdt)
    m1 = np.zeros(N_PAD, dtype=mdt)
    m0[:n_rows] = (T == 0).astype(mdt)
    m1[:n_rows] = (T == 1).astype(mdt)

    w1t = np.ascontiguousarray(np.asarray(W1, np.float32).T).astype(np.float16)
    w2s = np.zeros((HID, 32, 32), dtype=np.float16)
    w2v = np.asarray(W2, np.float32).reshape(HID).astype(np.float16)
    for c in range(32):
        w2s[:, c, c] = w2v
    w2s = w2s.reshape(HID, 32 * 32)
    b1h = np.asarray(b1, np.float32).reshape(HID).copy()
    b2h = np.asarray(b2, np.float32).reshape(1).copy()

    in_maps = []
    for cid in range(n_cores):
        in_maps.append(
            {
                "x": xd[cid * n_mega_core : (cid + 1) * n_mega_core],
                "m0": m0[cid * R_CORE : (cid + 1) * R_CORE],
                "m1": m1[cid * R_CORE : (cid + 1) * R_CORE],
                "w1t": w1t,
                "w2s": w2s,
                "b1": b1h,
                "b2": b2h,
            }
        )
    return in_maps


def run(x, T, W1, b1, W2, b2, dve_mod=DVE_MOD, f16_masks=True, inc_sums=True,
        rdma_ar=False, trace=False):
    in_maps = prep_inputs(x, T, W1, b1, W2, b2, f16_masks=f16_masks)
    nc = _get_nc(dve_mod, f16_masks, inc_sums, rdma_ar)
    res = run_bass_kernel_spmd(nc, in_maps, list(range(N_CORES)), trace=trace)
    out = np.concatenate([res.results[c]["out"] for c in range(N_CORES)])
    return out[: x.shape[0]].astype(np.float32, copy=False), res


def kernel(x, T, W1, b1, W2, b2):
    out, _ = run(x, T, W1, b1, W2, b2)
    return out


# revision 29
# speedup vs baseline: 1.1273x; 1.0717x over previous
"""
Trainium2 Bass kernel for DirectRankingModel:
    h = tanh(x @ W1.T + b1); s = (h @ W2.T + b2); e = exp(s)
    out = e / segment_sum(e, T)[T]    with 2 segments, N = 2,000,000 rows.

Strategy (8 NeuronCores, data-parallel over rows; measured 334 us vs 355 us
for the v1 baseline; the scalar engine's tanh stream is the bottleneck and
runs at ~100% occupancy during the main phase):
  - Host: cast x to fp16 and block-transpose into [nmega, 128, 2048] so each
    DMA moves 4KB contiguous runs. fp16 halves the HBM read (512->256 MB)
    vs the f32+cast-DMA baseline, taking DMA off the critical path.
    Host also builds fp16 masks m0/m1 = (T==0)/(T==1), zero on padded rows.
  - Device per core (R = 262144 rows; 4 super-tiles of 128 x 512 scores):
      * x loads ride the sync ring (the gpsimd SWDGE ring wedges the device
        if shared with remote-DMA descriptors; see rdma_ar note below).
      * mm1: two K=64 matmuls per 1024 rows (row-split PE), W1T stationary
        -> PSUM [128h, 1024r], 4 sub-tiles per 4096-row mega.
      * tanh on the scalar engine (fused +b1 bias), PSUM -> SBUF fp16.
        (A DVE/Pade offload was tried and measured SLOWER: no 2x DVE perf
        modes engage for tensor_tensor/stt on this silicon, so the 7-pass
        rational costs ~7.7 ns/elem vs scalar's 1.0 -- see dve_mod flag.)
      * mm2: 32 strip matrices [128, 32] with W2 embedded in column c
        accumulate block scores into PSUM [128, 512] per super-tile
        (Q=512 halves LDWEIGHTS vs Q=256). mm2 for mega k is emitted after
        mm1 of mega k+2 so the in-order PE queue never stalls on a late ht.
      * exp (+b2) -> E [128, 2048] f32 SBUF-resident; masked group sums run
        INCREMENTALLY per super-tile on the DVE (mul + reduce_sum into
        per-ST accumulator columns), overlapped under the next super-tile.
      * tail: 3 adds + ones-matmul partition reduce + 2-float HBM AllReduce
        (collective_compute; ~33 us -- an SBUF remote-DMA allreduce works
        standalone (probe verified, XOR-slot broadcast) but wedges the
        device inside this full kernel under every structure tried
        (tile_critical and raw post-context); kept behind rdma_ar=False), reciprocal, scale = inv0+m1*(inv1-inv0),
        out = E*scale, scatter-DMA out.
"""

import os
import sys

import numpy as np

for _p in ("/opt/trn_rl_repo", "/root/.axon_site/_ro/trn_rl_repo"):
    if os.path.isdir(_p) and _p not in sys.path:
        sys.path.insert(0, _p)

import concourse.bacc as bacc
import concourse.bass as bass
import concourse.tile as tile
from concourse import bass_isa, mybir
from concourse.bass_utils import run_bass_kernel_spmd

F16 = mybir.dt.float16
BF16 = mybir.dt.bfloat16
F32 = mybir.dt.float32
ALU = mybir.AluOpType
ACTF = mybir.ActivationFunctionType

N_CORES = 8
N_ROWS = 2_000_000
IN_DIM = 64
HID = 128

# Device-side geometry (per core).
Q = 512                 # rows per score-block (mm2 moving free dim)
N_ST = 4                # super-tiles per core; ST = 128 blocks x Q rows = 65536
MEGA_BLK = 32           # x blocks (128 rows each) per mega DMA tile -> 4096 rows
R_CORE = N_ST * 128 * Q          # 262144 rows per core
N_PAD = N_CORES * R_CORE         # 2097152 rows total (padded)

ROWS_MEGA = MEGA_BLK * 128       # 4096
HALF = ROWS_MEGA // 2            # 2048 rows per partition-half
N_MEGA_CORE = R_CORE // ROWS_MEGA            # 64
MEGAS_PER_ST = (128 * Q) // ROWS_MEGA        # 16
BLOCKS_PER_MEGA = ROWS_MEGA // Q             # 8

DVE_MOD = 0             # 1 of every DVE_MOD [128,2048] tanh chunks -> DVE path
DVE_PHASE = 1


def _ap(handle_ap, offset, dims):
    """Custom access pattern on a DRAM tensor: dims = [[step, count], ...]."""
    return bass.AP(tensor=handle_ap.tensor, offset=offset, ap=list(dims))


def build_nc(n_cores=N_CORES, dve_mod=DVE_MOD, f16_masks=True, inc_sums=True,
             rdma_ar=False):
    """Build the per-core Bass program (SPMD: same program, sliced inputs)."""
    from contextlib import ExitStack

    MDT = F16 if f16_masks else F32
    n_st = N_ST
    cols = n_st * Q            # E/mask/out columns per partition (2048)
    n_mega = N_MEGA_CORE

    nc = bacc.Bacc(num_devices=n_cores)

    x_in = nc.declare_dram_parameter(
        "x", [n_mega, 128, HALF], F16, isOutput=False
    )
    m0_in = nc.declare_dram_parameter("m0", [R_CORE], MDT, isOutput=False)
    m1_in = nc.declare_dram_parameter("m1", [R_CORE], MDT, isOutput=False)
    w1t_in = nc.declare_dram_parameter("w1t", [IN_DIM, HID], F16, isOutput=False)
    w2s_in = nc.declare_dram_parameter("w2s", [HID, 32 * 32], F16, isOutput=False)
    b1_in = nc.declare_dram_parameter("b1", [HID], F32, isOutput=False)
    b2_in = nc.declare_dram_parameter("b2", [1], F32, isOutput=False)
    out_t = nc.declare_dram_parameter("out", [R_CORE], F32, isOutput=True)
    gs_t = nc.declare_dram_parameter("gsums", [2], F32, isOutput=True)

    cc_in = nc.dram_tensor("cc_in", [2], F32)
    cc_out = nc.dram_tensor("cc_out", [2], F32, addr_space="Shared")
    cc_in_w = nc.dram_tensor("cc_in_w", [2], F32)
    cc_out_w = nc.dram_tensor("cc_out_w", [2], F32, addr_space="Shared")
    cc_in_w2 = nc.dram_tensor("cc_in_w2", [2], F32)
    cc_out_w2 = nc.dram_tensor("cc_out_w2", [2], F32, addr_space="Shared")

    with ExitStack() as ctx:
        tc = ctx.enter_context(tile.TileContext(nc))
        singles = ctx.enter_context(tc.tile_pool(name="singles", bufs=1))
        xx_pool = ctx.enter_context(tc.tile_pool(name="xx", bufs=3))
        ht_pool = ctx.enter_context(tc.tile_pool(name="ht", bufs=4))
        ph_pool = ctx.enter_context(tc.tile_pool(name="ph", bufs=3, space="PSUM"))
        ps_pool = ctx.enter_context(tc.tile_pool(name="ps", bufs=2, space="PSUM"))
        y_pool = ctx.enter_context(tc.tile_pool(name="y", bufs=2))
        dv_pool = ctx.enter_context(tc.tile_pool(name="dv", bufs=1))

        # ---- static setup ----------------------------------------------
        w1t_sb = singles.tile([128, HID], F16)     # both halves hold W1T
        nc.sync.dma_start(
            out=w1t_sb[:],
            in_=_ap(w1t_in[:], 0, [[0, 2], [HID, IN_DIM], [1, HID]]),
        )
        b1_sb = singles.tile([128, 1], F32)
        nc.sync.dma_start(out=b1_sb[:], in_=_ap(b1_in[:], 0, [[1, HID], [1, 1]]))
        b2_sb = singles.tile([128, 1], F32)
        nc.sync.dma_start(out=b2_sb[:], in_=_ap(b2_in[:], 0, [[0, 128], [1, 1]]))

        # 32 strip matrices [128, 32] fp16, strip c has W2 in column c.
        # (loaded after the first mega's work is queued -- see mega loop)
        strips = singles.tile([128, 32, 32], F16)

        # Masks fp16, E f32, persistent SBUF tiles.
        m0_sb = singles.tile([128, cols], MDT)
        m1_sb = singles.tile([128, cols], MDT)
        mask_dims = [[Q, 128], [128 * Q, n_st], [1, Q]]
        e_sb = singles.tile([128, cols], F32)
        scratch = singles.tile([128, Q], F32)      # ttr full-tensor out (unused)
        scale_sb = singles.tile([128, cols], F32)
        out_sb = singles.tile([128, cols], F32)
        rr_accs = [
            singles.tile([128, 2], F32, name=f"rr_acc{i}") for i in range(2)
        ]
        rr_red = singles.tile([128, 2], F32)
        acc_sb = singles.tile([128, 2 * n_st], F32)
        ones_sb = singles.tile([128, 1], F32)
        nc.vector.memset(ones_sb[:], 1.0)
        ones_row = singles.tile([1, 128], F32)
        nc.vector.memset(ones_row[:], 1.0)
        g_sb = singles.tile([128, 2], F32)
        inv = singles.tile([128, 2], F32)
        dinv = singles.tile([128, 1], F32)

        # ---- phase 1: matmuls / tanh (scalar + DVE split) / scores / exp
        # mm2 for mega k is emitted after mm1 of mega k+LOOKAHEAD so a slow
        # DVE tanh chunk cannot stall the in-order PE queue.
        LOOKAHEAD = 2
        ht_tiles = {}
        s_ps_tiles = {}

        def emit_mm1(mega):
            xx = xx_pool.tile([128, HALF], F16, tag="xx")
            src = _ap(x_in[:], mega * 128 * HALF, [[HALF, 128], [1, HALF]])
            # sync ring: the gpsimd SWDGE ring is reserved for the RDMA
            # allreduce (sharing it with dma_start wedges the device)
            nc.sync.dma_start(out=xx[:], in_=src)

            ht = ht_pool.tile([128, ROWS_MEGA], F16, tag="ht")
            ht_tiles[mega] = ht
            for half in range(2):
                chunk = mega * 2 + half
                is_dve = dve_mod > 0 and (chunk % dve_mod) == DVE_PHASE
                if is_dve:
                    y = y_pool.tile([128, 2048], BF16, tag="y")
                for t2 in range(2):
                    t = half * 2 + t2
                    ph = ph_pool.tile([128, 1024], F32, tag="ph")
                    nc.tensor.matmul(
                        ph[:, 0:512],
                        w1t_sb[0:64, :],
                        xx[0:64, t * 512 : (t + 1) * 512],
                        start=True,
                        stop=True,
                    )
                    nc.tensor.matmul(
                        ph[:, 512:1024],
                        w1t_sb[64:128, :],
                        xx[64:128, t * 512 : (t + 1) * 512],
                        start=True,
                        stop=True,
                    )
                    # ht col layout is (t, g, j): col = t*1024 + g*512 + j,
                    # holding row mega_base + g*2048 + t*512 + j.
                    if is_dve:
                        # GPSIMD cannot read PSUM; bias+cast runs on DVE
                        nc.vector.tensor_scalar(
                            out=y[:, t2 * 1024 : (t2 + 1) * 1024],
                            in0=ph[:, 0:1024],
                            scalar1=b1_sb[:],
                            scalar2=None,
                            op0=ALU.add,
                        )
                    else:
                        nc.scalar.activation(
                            out=ht[:, t * 1024 : (t + 1) * 1024],
                            in_=ph[:, 0:1024],
                            func=ACTF.Tanh,
                            bias=b1_sb[:],
                            scale=1.0,
                        )
                if is_dve:
                    # Pade(5,4): h = y*(945+105u+u^2) / (945+420u+15u^2)
                    u = dv_pool.tile([128, 2048], BF16, tag="u")
                    n1 = dv_pool.tile([128, 2048], BF16, tag="n1")
                    nx = dv_pool.tile([128, 2048], BF16, tag="nx")
                    d1 = dv_pool.tile([128, 2048], BF16, tag="d1")
                    d2 = dv_pool.tile([128, 2048], F32, tag="d2")
                    rcp = dv_pool.tile([128, 2048], F32, tag="rcp")
                    nc.vector.tensor_tensor(
                        out=u[:], in0=y[:], in1=y[:], op=ALU.mult
                    )
                    nc.vector.scalar_tensor_tensor(
                        out=n1[:], in0=u[:], scalar=105.0, in1=u[:],
                        op0=ALU.add, op1=ALU.mult,
                    )
                    nc.vector.scalar_tensor_tensor(
                        out=nx[:], in0=n1[:], scalar=945.0, in1=y[:],
                        op0=ALU.add, op1=ALU.mult,
                    )
                    nc.vector.scalar_tensor_tensor(
                        out=d1[:], in0=u[:], scalar=28.0, in1=u[:],
                        op0=ALU.add, op1=ALU.mult,
                    )
                    nc.vector.tensor_scalar(
                        out=d2[:], in0=d1[:], scalar1=63.0, scalar2=15.0,
                        op0=ALU.add, op1=ALU.mult,
                    )
                    nc.vector.reciprocal_approx_fast(out=rcp[:], in_=d2[:])
                    ho = half * 2048
                    # final multiply on GPSIMD (SBUF-only) to unload DVE
                    nc.gpsimd.tensor_tensor(
                        out=ht[:, ho : ho + 2048], in0=nx[:], in1=rcp[:],
                        op=ALU.mult,
                    )

        def emit_mm2(mega):
            st = mega // MEGAS_PER_ST
            if mega % MEGAS_PER_ST == 0:
                s_ps_tiles[st] = ps_pool.tile(
                    [128, Q], F32, tag="score", name=f"s_ps{st}"
                )
            s_ps = s_ps_tiles[st]
            ht = ht_tiles.pop(mega)
            for b8 in range(BLOCKS_PER_MEGA):
                B = mega * BLOCKS_PER_MEGA + b8
                c = B % 32
                g = (B // 32) % 4
                hoff = (b8 % 4) * 1024 + (b8 // 4) * 512
                nc.tensor.matmul(
                    s_ps[32 * g : 32 * g + 32, :],
                    strips[:, c, :],
                    ht[:, hoff : hoff + Q],
                    start=(c == 0),
                    stop=(c == 31),
                    skip_group_check=True,
                    tile_position=(0, 32 * g),
                )
            if mega % MEGAS_PER_ST != MEGAS_PER_ST - 1:
                return
            # super-tile closed: exp + incremental masked sums
            s_ps = s_ps_tiles.pop(st)
            nc.scalar.activation(
                out=e_sb[:, st * Q : (st + 1) * Q],
                in_=s_ps[:],
                func=ACTF.Exp,
                bias=b2_sb[:],
                scale=1.0,
            )
            if inc_sums:
                e_sl = e_sb[:, st * Q : (st + 1) * Q]
                for gi, m_sb in enumerate((m0_sb, m1_sb)):
                    nc.vector.tensor_mul(
                        scratch[:], e_sl, m_sb[:, st * Q : (st + 1) * Q]
                    )
                    nc.vector.reduce_sum(
                        acc_sb[:, 2 * st + gi : 2 * st + gi + 1],
                        scratch[:],
                        axis=mybir.AxisListType.X,
                    )

        for mega in range(n_mega):
            emit_mm1(mega)
            if mega == 0:
                if not rdma_ar:
                    # warmup AllReduces on dummy data: pay the CC-core setup
                    # cost during phase 1 so the real tail collective is hot
                    for wi, wo in ((cc_in_w, cc_out_w), (cc_in_w2, cc_out_w2)):
                        nc.gpsimd.collective_compute(
                            "AllReduce",
                            ALU.add,
                            replica_groups=[list(range(n_cores))],
                            ins=[wi[:]],
                            outs=[wo[:]],
                        )
                # deferred setup loads: off the first-tanh critical path
                nc.sync.dma_start(
                    out=strips[:],
                    in_=_ap(w2s_in[:], 0, [[32 * 32, HID], [1, 32 * 32]]),
                )
                nc.sync.dma_start(out=m0_sb[:], in_=_ap(m0_in[:], 0, mask_dims))
                nc.sync.dma_start(out=m1_sb[:], in_=_ap(m1_in[:], 0, mask_dims))
            if mega >= LOOKAHEAD:
                emit_mm2(mega - LOOKAHEAD)
        for mega in range(n_mega - LOOKAHEAD, n_mega):
            emit_mm2(mega)

        # ---- partition reduce + allreduce ------------------------------
        if inc_sums:
            rr_fin = rr_accs[1]
            nc.vector.tensor_add(rr_accs[0][:], acc_sb[:, 0:2], acc_sb[:, 2:4])
            nc.vector.tensor_add(rr_red[:], acc_sb[:, 4:6], acc_sb[:, 6:8])
            nc.vector.tensor_add(rr_fin[:], rr_accs[0][:], rr_red[:])
        else:
            rr_fin = rr_accs[0]
            nc.vector.tensor_mul(scale_sb[:], e_sb[:], m0_sb[:])
            nc.vector.reduce_sum(
                rr_fin[:, 0:1], scale_sb[:], axis=mybir.AxisListType.X
            )
            nc.vector.tensor_mul(scale_sb[:], e_sb[:], m1_sb[:])
            nc.vector.reduce_sum(
                rr_fin[:, 1:2], scale_sb[:], axis=mybir.AxisListType.X
            )
        if rdma_ar:
            # Partition-reduce the per-partition partials to [1, 2] and
            # broadcast back to all 128 partitions, so the cross-core
            # exchange operand is partition-uniform and the post-exchange
            # tail is pure vector work.
            rr_all = singles.tile([128, 16], F32)
            tmp8 = singles.tile([128, 8], F32)
            tmp4 = singles.tile([128, 4], F32)
            rr_g = singles.tile([128, 2], F32)
            rr_bcast = singles.tile([128, 2], F32)
            ps_rr = ps_pool.tile([128, Q], F32, tag="score")
            nc.tensor.matmul(
                ps_rr[0:1, 0:2], ones_sb[:], rr_fin[:], start=True, stop=True
            )
            nc.scalar.activation(
                out=rr_red[0:1, :],
                in_=ps_rr[0:1, 0:2],
                func=ACTF.Copy,
                bias=0.0,
                scale=1.0,
            )
            ps_bc = ps_pool.tile([128, Q], F32, tag="score", name="ps_bc")
            nc.tensor.matmul(
                ps_bc[:, 0:2],
                ones_row[:],
                rr_red[0:1, 0:2],
                start=True,
                stop=True,
            )
            nc.vector.tensor_copy(rr_bcast[:], ps_bc[:, 0:2])
        else:
            ps_rr = ps_pool.tile([128, Q], F32, tag="score")
            nc.tensor.matmul(
                ps_rr[0:1, 0:2], ones_sb[:], rr_fin[:], start=True, stop=True
            )
            nc.scalar.activation(
                out=rr_red[0:1, :],
                in_=ps_rr[0:1, 0:2],
                func=ACTF.Copy,
                bias=0.0,
                scale=1.0,
            )
            nc.gpsimd.dma_start(out=cc_in[:], in_=rr_red[0:1, :])
            nc.gpsimd.collective_compute(
                "AllReduce",
                ALU.add,
                replica_groups=[list(range(n_cores))],
                ins=[cc_in[:]],
                outs=[cc_out[:]],
            )
            nc.sync.dma_start(out=gs_t[:], in_=cc_out[:])
            # broadcast the 2 global sums to all 128 partitions on-chip
            # (a stride-0 DRAM broadcast DMA costs 128 tiny packets)
            nc.sync.dma_start(
                out=rr_red[0:1, 0:2], in_=_ap(cc_out[:], 0, [[2, 1], [1, 2]])
            )
            ps_bc = ps_pool.tile([128, Q], F32, tag="score", name="ps_bc")
            nc.tensor.matmul(
                ps_bc[:, 0:2],
                ones_row[:],
                rr_red[0:1, 0:2],
                start=True,
                stop=True,
            )

        if not rdma_ar:
            # ---- normalize + store (tile-scheduled, 2 halves so the out
            # DMA of half 0 overlaps half 1's vector work) ----------------
            nc.vector.reciprocal(out=inv[:], in_=ps_bc[:, 0:2])
            nc.vector.tensor_sub(dinv[:], inv[:, 1:2], inv[:, 0:1])
            hc = cols // 2
            for h in range(2):
                sl = slice(h * hc, (h + 1) * hc)
                nc.vector.tensor_scalar(
                    out=scale_sb[:, sl],
                    in0=m1_sb[:, sl],
                    scalar1=dinv[:],
                    scalar2=inv[:, 0:1],
                    op0=ALU.mult,
                    op1=ALU.add,
                )
                nc.vector.tensor_mul(
                    out_sb[:, sl], scale_sb[:, sl], e_sb[:, sl]
                )
                nc.sync.dma_start(
                    out=_ap(
                        out_t[:],
                        h * (128 * Q * n_st // 2),
                        [[Q, 128], [128 * Q, n_st // 2], [1, Q]],
                    ),
                    in_=out_sb[:, sl],
                )

    if rdma_ar:
        # ---- raw-bass tail after the TileContext teardown barrier ------
        # SBUF-to-SBUF allreduce: 8 broadcasts, each with ONE real
        # destination at XOR slot k (receiver r's slot k <- sender r^k;
        # identical SPMD program on every core). All cross-engine ordering
        # is manual; semaphores pinned far above the tile range. Tile APs
        # are rebuilt over the concrete SBUF tensors (the tile symbolic
        # form cannot lower outside the TileContext).
        def P(sym):
            return bass.AP(
                tensor=sym.tensor.concrete_tensor(),
                offset=sym.offset,
                ap=[list(d) for d in sym.ap],
            )

        rsem = nc.alloc_semaphore(name="ar_rsem", num=244)
        lsem = nc.alloc_semaphore(name="ar_lsem", num=245)
        psem = nc.alloc_semaphore(name="ar_psem", num=246)
        fsem = nc.alloc_semaphore(name="ar_fsem", num=247)
        dsem = nc.alloc_semaphore(name="ar_dsem", num=248)
        for k in range(n_cores):
            rdests = [None] * n_cores
            rdests[k] = (0, k)
            nc.gpsimd.remote_dma_broadcast(
                out_ap=P(rr_all[:, 2 * k : 2 * k + 2]),
                in_ap=P(rr_bcast[:]),
                remote_sem=rsem,
                local_sem=lsem,
                rdests=rdests,
            ).then_inc(psem, 1)
        nc.gpsimd.wait_ge(psem, n_cores)
        nc.gpsimd.trigger_dma(count=n_cores)
        # each of the 8 senders' arrivals bumps rsem by 16//8 = 2
        nc.vector.wait_ge(rsem, 2 * n_cores)
        for i in range(4):
            nc.vector.tensor_add(
                P(tmp8[:, 2 * i : 2 * i + 2]),
                P(rr_all[:, 2 * i : 2 * i + 2]),
                P(rr_all[:, 2 * i + 8 : 2 * i + 10]),
            )
        nc.vector.tensor_add(P(tmp4[:, 0:2]), P(tmp8[:, 0:2]), P(tmp8[:, 4:6]))
        nc.vector.tensor_add(P(tmp4[:, 2:4]), P(tmp8[:, 2:4]), P(tmp8[:, 6:8]))
        nc.vector.tensor_add(P(rr_g[:]), P(tmp4[:, 0:2]), P(tmp4[:, 2:4]))
        # normalize + store, all on vector then sync
        nc.vector.reciprocal(out=P(inv[:]), in_=P(rr_g[:]))
        nc.vector.tensor_sub(P(dinv[:]), P(inv[:, 1:2]), P(inv[:, 0:1]))
        nc.vector.tensor_scalar(
            out=P(scale_sb[:]),
            in0=P(m1_sb[:]),
            scalar1=P(dinv[:]),
            scalar2=P(inv[:, 0:1]),
            op0=ALU.mult,
            op1=ALU.add,
        )
        nc.vector.tensor_mul(
            P(out_sb[:]), P(scale_sb[:]), P(e_sb[:])
        ).then_inc(fsem, 1)
        nc.sync.wait_ge(fsem, 1)
        nc.sync.dma_start(out=gs_t[:], in_=P(rr_g[0:1, 0:2])).then_inc(
            dsem, 16
        )
        nc.sync.dma_start(
            out=_ap(out_t[:], 0, [[Q, 128], [128 * Q, n_st], [1, Q]]),
            in_=P(out_sb[:]),
        ).then_inc(dsem, 16)
        # flush everything, then clear our semaphores for re-execution
        nc.gpsimd.wait_ge(lsem, 16 * n_cores)
        nc.gpsimd.wait_ge(dsem, 32)
        for s in (rsem, lsem, psem, fsem, dsem):
            nc.gpsimd.sem_clear(s)

    nc.compile()
    return nc


_NC_CACHE = {}


def _get_nc(dve_mod=DVE_MOD, f16_masks=True, inc_sums=True, rdma_ar=False):
    key = (dve_mod, f16_masks, inc_sums, rdma_ar)
    if key not in _NC_CACHE:
        _NC_CACHE[key] = build_nc(
            dve_mod=dve_mod, f16_masks=f16_masks, inc_sums=inc_sums,
            rdma_ar=rdma_ar,
        )
    return _NC_CACHE[key]


def prep_inputs(x, T, W1, b1, W2, b2, n_cores=N_CORES, f16_masks=True):
    """Host-side shard/layout prep -> per-core input maps."""
    n_rows = x.shape[0]
    mdt = np.float16 if f16_masks else np.float32

    x16 = np.asarray(x, dtype=np.float32).astype(np.float16)
    n_mega_tot = N_PAD // ROWS_MEGA
    n_full = n_rows // ROWS_MEGA
    xd = np.zeros((n_mega_tot, 128, HALF), dtype=np.float16)
    xd[:n_full] = (
        x16[: n_full * ROWS_MEGA]
        .reshape(n_full, 2, HALF, IN_DIM)
        .transpose(0, 1, 3, 2)
        .reshape(n_full, 128, HALF)
    )
    rem = n_rows - n_full * ROWS_MEGA
    if rem:
        r0 = min(rem, HALF)
        xd[n_full, :IN_DIM, :r0] = x16[n_full * ROWS_MEGA :][:r0].T
        if rem > HALF:
            xd[n_full, IN_DIM:, : rem - HALF] = x16[n_full * ROWS_MEGA + HALF :].T
    n_mega_core = n_mega_tot // n_cores

    T = np.asarray(T)
    m0 = np.zeros(N_PAD, dtype=mdt)
    m1 = np.zeros(N_PAD, dtype=mdt)
    m0[:n_rows] = (T == 0).astype(mdt)
    m1[:n_rows] = (T == 1).astype(mdt)

    w1t = np.ascontiguousarray(np.asarray(W1, np.float32).T).astype(np.float16)
    w2s = np.zeros((HID, 32, 32), dtype=np.float16)
    w2v = np.asarray(W2, np.float32).reshape(HID).astype(np.float16)
    for c in range(32):
        w2s[:, c, c] = w2v
    w2s = w2s.reshape(HID, 32 * 32)
    b1h = np.asarray(b1, np.float32).reshape(HID).copy()
    b2h = np.asarray(b2, np.float32).reshape(1).copy()

    in_maps = []
    for cid in range(n_cores):
        in_maps.append(
            {
                "x": xd[cid * n_mega_core : (cid + 1) * n_mega_core],
                "m0": m0[cid * R_CORE : (cid + 1) * R_CORE],
                "m1": m1[cid * R_CORE : (cid + 1) * R_CORE],
                "w1t": w1t,
                "w2s": w2s,
                "b1": b1h,
                "b2": b2h,
            }
        )
    return in_maps


def run(x, T, W1, b1, W2, b2, dve_mod=DVE_MOD, f16_masks=True, inc_sums=True,
        rdma_ar=False, trace=False):
    in_maps = prep_inputs(x, T, W1, b1, W2, b2, f16_masks=f16_masks)
    nc = _get_nc(dve_mod, f16_masks, inc_sums, rdma_ar)
    res = run_bass_kernel_spmd(nc, in_maps, list(range(N_CORES)), trace=trace)
    out = np.concatenate([res.results[c]["out"] for c in range(N_CORES)])
    return out[: x.shape[0]].astype(np.float32, copy=False), res


def kernel(x, T, W1, b1, W2, b2):
    out, _ = run(x, T, W1, b1, W2, b2)
    return out
